# revision 2
# baseline (speedup 1.0000x reference)
"""Trainium2 Bass kernel for batched global mean pooling (segment mean).

Computes, for 2M sorted nodes with 64 features and 1024 graphs:
    out[g, f] = mean over nodes n with batch[n] == g of node_features[n, f]
(empty graphs -> zeros), distributed over 8 NeuronCores.

Strategy (graph sharding; no collectives):
  - Core k owns graphs [128k, 128k+128). batch is sorted, so each graph's
    nodes are a contiguous row range of node_features.
  - Host (inside kernel(), per call) lays out each core's nodes on a
    [128, T] grid: partition p gets only the nodes of local graph p,
    zero-padded to T = max graph size. Column 64 carries a 1.0/0.0
    valid-node flag (the count column).
  - Features are split into fp16 (hi, lo) pairs so the PE runs at full
    rate (fp32 matmul is 4x slower); hi + lo recovers fp32 precision
    since products accumulate into fp32 PSUM.
  - Device: each matmul is identity128.T @ slab for a [128, 7*65] fp16
    slab, accumulating into one [128, 455] f32 PSUM bank: partition =
    local graph. After all chunks: fold the 7 column blocks, divide by
    max(count, 1), DMA the [128, 64] result out.
  - Host concatenates the 8 per-core [128, 64] outputs.

The Bass program is compiled per call with the chunk count derived from
the actual input, so any node/graph distribution is handled.
"""

import math

import numpy as np

import concourse.mybir as mybir
import concourse.tile as tile
from concourse import bacc
from concourse.bass_utils import run_bass_kernel_spmd

NCORES = 8
P = 128  # partitions = local graphs per core
F = 64  # features
FC = F + 1  # features + count column
B = 7  # tiles (node-rows) per matmul: N = 7*65 = 455 <= 512 fp32 PSUM bank
MM_PER_CHUNK = 9  # matmuls per DMA chunk
TB = B * MM_PER_CHUNK  # 63 tiles per chunk -> ~1.05 MB per DMA

# set by tests to capture a profile; harness path leaves these alone
TRACE = False
LAST_RESULTS = None


def _build(n_chunks: int):
    nc = bacc.Bacc("TRN2", target_bir_lowering=False, debug=False, num_devices=NCORES)
    hi = nc.dram_tensor(
        "hi", [n_chunks, P, TB * FC], mybir.dt.float16, kind="ExternalInput"
    ).ap()
    lo = nc.dram_tensor(
        "lo", [n_chunks, P, TB * FC], mybir.dt.float16, kind="ExternalInput"
    ).ap()
    ident = nc.dram_tensor("ident", [P, P], mybir.dt.float16, kind="ExternalInput").ap()
    out = nc.dram_tensor("out", [P, F], mybir.dt.float32, kind="ExternalOutput").ap()

    n_mm = n_chunks * 2 * MM_PER_CHUNK
    with tile.TileContext(nc) as tc:
        with (
            tc.tile_pool(name="consts", bufs=1) as consts,
            tc.tile_pool(name="io", bufs=3) as io,
            tc.tile_pool(name="ep", bufs=1) as ep,
            tc.tile_pool(name="acc", bufs=1, space="PSUM") as accp,
        ):
            ident_sb = consts.tile([P, P], mybir.dt.float16)
            nc.sync.dma_start(ident_sb[:], ident[:])

            psum = accp.tile([P, B * FC], mybir.dt.float32)
            mm = 0
            for c in range(n_chunks):
                hi_t = io.tile([P, TB * FC], mybir.dt.float16, tag="hi")
                nc.sync.dma_start(hi_t[:], hi[c, :, :])
                lo_t = io.tile([P, TB * FC], mybir.dt.float16, tag="lo")
                nc.sync.dma_start(lo_t[:], lo[c, :, :])
                for t in (hi_t, lo_t):
                    for b in range(MM_PER_CHUNK):
                        nc.tensor.matmul(
                            psum[:],
                            ident_sb[:],
                            t[:, b * B * FC : (b + 1) * B * FC],
                            start=(mm == 0),
                            stop=(mm == n_mm - 1),
                        )
                        mm += 1

            # fold the B column blocks: s[:, 0:65] = sum_b psum[:, b*65:(b+1)*65]
            # (DVE may read at most one PSUM operand per instruction)
            s = ep.tile([P, FC], mybir.dt.float32)
            nc.vector.tensor_copy(s[:], psum[:, 0:FC])
            for b in range(1, B):
                nc.vector.tensor_add(s[:], s[:], psum[:, b * FC : (b + 1) * FC])

            safe = ep.tile([P, 1], mybir.dt.float32)
            recip = ep.tile([P, 1], mybir.dt.float32)
            res = ep.tile([P, F], mybir.dt.float32)
            nc.vector.tensor_scalar_max(safe[:], s[:, F : F + 1], 1.0)
            nc.vector.reciprocal(recip[:], safe[:])
            nc.vector.tensor_scalar_mul(res[:], s[:, 0:F], recip[:])
            nc.sync.dma_start(out[:], res[:])

    nc.compile()
    return nc


def kernel(node_features, batch, num_graphs):
    global LAST_RESULTS
    x = np.asarray(node_features, dtype=np.float32)
    b = np.asarray(batch, dtype=np.int64).ravel()
    G = int(num_graphs)
    N = x.shape[0]
    assert x.shape[1] == F, f"expected {F} features, got {x.shape[1]}"

    if not np.all(b[1:] >= b[:-1]):  # defensive: layout relies on sorted batch
        order = np.argsort(b, kind="stable")
        b = b[order]
        x = x[order]

    gpc = math.ceil(G / NCORES)  # local graphs per core
    assert gpc <= P, f"num_graphs {G} too large for {NCORES} cores x {P} partitions"

    counts = np.bincount(b, minlength=NCORES * gpc).astype(np.int64)
    starts = np.zeros(NCORES * gpc + 1, dtype=np.int64)
    np.cumsum(counts, out=starts[1:])
    t_max = int(counts.max()) if N else 1
    n_chunks = max(1, math.ceil(t_max / TB))
    t_cap = n_chunks * TB

    x_ext = np.vstack([x, np.zeros((1, F), dtype=np.float32)])  # row N = zeros
    col = np.arange(t_cap, dtype=np.int64)
    ident = np.eye(P, dtype=np.float16)

    in_maps = []
    for k in range(NCORES):
        g0 = k * gpc
        cg = counts[g0 : g0 + gpc]
        sg = starts[g0 : g0 + gpc]
        valid = col[None, :] < cg[:, None]  # [gpc, t_cap]
        idx = np.where(valid, sg[:, None] + col[None, :], N)
        if gpc < P:  # pad partitions for graph counts not divisible by 8
            pad = np.full((P - gpc, t_cap), N, dtype=np.int64)
            idx = np.vstack([idx, pad])
            valid = np.vstack([valid, np.zeros((P - gpc, t_cap), dtype=bool)])

        feats = x_ext[idx]  # [P, t_cap, F] f32
        hi16 = feats.astype(np.float16)
        lo16 = (feats - hi16.astype(np.float32)).astype(np.float16)

        hi_arr = np.empty((P, t_cap, FC), dtype=np.float16)
        hi_arr[:, :, :F] = hi16
        hi_arr[:, :, F] = valid
        lo_arr = np.empty((P, t_cap, FC), dtype=np.float16)
        lo_arr[:, :, :F] = lo16
        lo_arr[:, :, F] = 0.0

        # device DMA layout: [chunk, partition, tile-within-chunk * FC]
        def to_dev(a):
            return np.ascontiguousarray(
                a.reshape(P, n_chunks, TB * FC).transpose(1, 0, 2)
            )

        in_maps.append({"hi": to_dev(hi_arr), "lo": to_dev(lo_arr), "ident": ident})

    nc = _build(n_chunks)
    res = run_bass_kernel_spmd(nc, in_maps, core_ids=list(range(NCORES)), trace=TRACE)
    LAST_RESULTS = res

    out = np.concatenate([res.results[k]["out"] for k in range(NCORES)], axis=0)
    return out[:G]


# revision 3
# speedup vs baseline: 1.0288x; 1.0288x over previous
"""Trainium2 Bass kernel for batched global mean pooling (segment mean).

Computes, for N sorted nodes with 64 features and G graphs:
    out[g, f] = mean over nodes n with batch[n] == g of node_features[n, f]
(empty graphs -> zeros), distributed over 8 NeuronCores.

Strategy (graph sharding; no collectives):
  - Core k owns graphs [128k, 128(k+1)). batch is sorted, so each graph's
    nodes are a contiguous row range of node_features.
  - Host (inside kernel(), per call) lays out each core's nodes on a
    [128, T] grid: partition p gets only the nodes of local graph p,
    zero-padded to T = max graph size.
  - Features are split into fp16 (hi, lo) pairs so the PE runs at full
    rate (fp32 matmul is 4x slower); hi + lo recovers fp32 precision
    since the products accumulate into fp32 PSUM.
  - Device: each matmul is identity128.T @ slab for a [128, 7*64] fp16
    slab, accumulating into one [128, 448] f32 PSUM bank: partition =
    local graph. After all chunks: fold the 7 column blocks, multiply by
    host-provided 1/max(count, 1), DMA the [128, 64] result out.
  - Host concatenates the 8 per-core [128, 64] outputs.

The Bass program is compiled per call with the chunk count derived from
the actual input, so any node/graph distribution is handled.
"""

import math

import numpy as np

import concourse.mybir as mybir
import concourse.tile as tile
from concourse import bacc
from concourse.bass_utils import run_bass_kernel_spmd

NCORES = 8
P = 128  # partitions = local graphs per core
F = 64  # features
B = 7  # tiles (node-rows) per matmul: N = 7*64 = 448 <= 512 f32 PSUM bank
TB = 63  # tiles per full DMA chunk (~1.03 MB per chunk)

# set by tests to capture a profile; harness path leaves these alone
TRACE = False
LAST_RESULTS = None


def _chunks(t_cap):
    """Split t_cap tiles into DMA chunks: full 63-tile chunks + remainder."""
    out = []
    t = 0
    while t < t_cap:
        n = min(TB, t_cap - t)
        out.append((t, n))
        t += n
    return out


def _build(t_cap):
    nc = bacc.Bacc("TRN2", target_bir_lowering=False, debug=False, num_devices=NCORES)
    hi = nc.dram_tensor("hi", [P, t_cap * F], mybir.dt.float16, kind="ExternalInput").ap()
    lo = nc.dram_tensor("lo", [P, t_cap * F], mybir.dt.float16, kind="ExternalInput").ap()
    ident = nc.dram_tensor("ident", [P, P], mybir.dt.float16, kind="ExternalInput").ap()
    inv = nc.dram_tensor("inv", [P, 1], mybir.dt.float32, kind="ExternalInput").ap()
    out = nc.dram_tensor("out", [P, F], mybir.dt.float32, kind="ExternalOutput").ap()

    chunks = _chunks(t_cap)
    n_mm = 2 * (t_cap // B)
    with tile.TileContext(nc) as tc:
        with (
            tc.tile_pool(name="consts", bufs=1) as consts,
            tc.tile_pool(name="io", bufs=3) as io,
            tc.tile_pool(name="ep", bufs=1) as ep,
            tc.tile_pool(name="acc", bufs=1, space="PSUM") as accp,
        ):
            ident_sb = consts.tile([P, P], mybir.dt.float16)
            nc.sync.dma_start(ident_sb[:], ident[:])
            inv_sb = consts.tile([P, 1], mybir.dt.float32)
            nc.sync.dma_start(inv_sb[:], inv[:])

            psum = accp.tile([P, B * F], mybir.dt.float32)
            mm = 0
            for t0, nt in chunks:
                hi_t = io.tile([P, TB * F], mybir.dt.float16, tag="hi")
                nc.sync.dma_start(hi_t[:, : nt * F], hi[:, t0 * F : (t0 + nt) * F])
                lo_t = io.tile([P, TB * F], mybir.dt.float16, tag="lo")
                nc.sync.dma_start(lo_t[:, : nt * F], lo[:, t0 * F : (t0 + nt) * F])
                for t in (hi_t, lo_t):
                    for b in range(nt // B):
                        nc.tensor.matmul(
                            psum[:],
                            ident_sb[:],
                            t[:, b * B * F : (b + 1) * B * F],
                            start=(mm == 0),
                            stop=(mm == n_mm - 1),
                        )
                        mm += 1
            assert mm == n_mm

            # fold the B column blocks: s = sum_b psum[:, b*64:(b+1)*64]
            # (DVE may read at most one PSUM operand per instruction)
            s = ep.tile([P, F], mybir.dt.float32)
            nc.vector.tensor_copy(s[:], psum[:, 0:F])
            for b in range(1, B):
                nc.vector.tensor_add(s[:], s[:], psum[:, b * F : (b + 1) * F])

            res = ep.tile([P, F], mybir.dt.float32)
            nc.vector.tensor_scalar_mul(res[:], s[:], inv_sb[:])
            nc.sync.dma_start(out[:], res[:])

    nc.compile()
    return nc


def kernel(node_features, batch, num_graphs):
    global LAST_RESULTS
    x = np.asarray(node_features, dtype=np.float32)
    b = np.asarray(batch, dtype=np.int64).ravel()
    G = int(num_graphs)
    N = x.shape[0]
    assert x.shape[1] == F, f"expected {F} features, got {x.shape[1]}"

    if not np.all(b[1:] >= b[:-1]):  # defensive: layout relies on sorted batch
        order = np.argsort(b, kind="stable")
        b = b[order]
        x = x[order]

    gpc = math.ceil(G / NCORES)  # local graphs per core
    assert gpc <= P, f"num_graphs {G} too large for {NCORES} cores x {P} partitions"

    counts = np.bincount(b, minlength=NCORES * gpc).astype(np.int64)
    starts = np.zeros(NCORES * gpc + 1, dtype=np.int64)
    np.cumsum(counts, out=starts[1:])
    t_max = int(counts.max()) if N else 1
    t_cap = max(B, math.ceil(t_max / B) * B)

    x_ext = np.vstack([x, np.zeros((1, F), dtype=np.float32)])  # row N = zeros
    col = np.arange(t_cap, dtype=np.int64)
    ident = np.eye(P, dtype=np.float16)

    in_maps = []
    for k in range(NCORES):
        g0 = k * gpc
        cg = counts[g0 : g0 + gpc]
        sg = starts[g0 : g0 + gpc]
        valid = col[None, :] < cg[:, None]  # [gpc, t_cap]
        idx = np.where(valid, sg[:, None] + col[None, :], N)
        if gpc < P:  # pad partitions when graph count is not divisible by 8
            idx = np.vstack([idx, np.full((P - gpc, t_cap), N, dtype=np.int64)])

        feats = x_ext[idx]  # [P, t_cap, F] f32
        hi16 = np.ascontiguousarray(feats.astype(np.float16).reshape(P, t_cap * F))
        lo16 = np.ascontiguousarray(
            (feats - hi16.reshape(P, t_cap, F).astype(np.float32))
            .astype(np.float16)
            .reshape(P, t_cap * F)
        )

        inv = np.zeros((P, 1), dtype=np.float32)
        inv[:gpc, 0] = 1.0 / np.maximum(cg, 1)
        in_maps.append({"hi": hi16, "lo": lo16, "ident": ident, "inv": inv})

    nc = _build(t_cap)
    res = run_bass_kernel_spmd(nc, in_maps, core_ids=list(range(NCORES)), trace=TRACE)
    LAST_RESULTS = res

    out = np.concatenate([res.results[k]["out"] for k in range(NCORES)], axis=0)
    return out[:G]


# revision 5
# speedup vs baseline: 1.0291x; 1.0003x over previous
"""Trainium2 Bass kernel for batched global mean pooling (segment mean).

Computes, for N sorted nodes with 64 features and G graphs:
    out[g, f] = mean over nodes n with batch[n] == g of node_features[n, f]
(empty graphs -> zeros), distributed over 8 NeuronCores.

Strategy (graph sharding; no collectives):
  - Core k owns graphs [128k, 128(k+1)). batch is sorted, so each graph's
    nodes are a contiguous row range of node_features.
  - Host (inside kernel(), per call) lays out each core's nodes on a
    [128, T] grid: partition p gets only the nodes of local graph p,
    zero-padded to T = max graph size.
  - Features are split into fp16 (hi, lo) pairs so the PE runs at full
    rate (fp32 matmul is 4x slower); hi + lo recovers fp32 precision
    since the products accumulate into fp32 PSUM.
  - Device: each matmul is identity128.T @ slab for a [128, 7*64] fp16
    slab, accumulating into one [128, 448] f32 PSUM bank: partition =
    local graph. After all chunks: fold the 7 column blocks, multiply by
    host-provided 1/max(count, 1), DMA the [128, 64] result out.
  - Host concatenates the 8 per-core [128, 64] outputs.

The Bass program is compiled per call with the chunk count derived from
the actual input, so any node/graph distribution is handled.
"""

import math

import numpy as np

import concourse.mybir as mybir
import concourse.tile as tile
from concourse import bacc
from concourse.bass_utils import run_bass_kernel_spmd

NCORES = 8
P = 128  # partitions = local graphs per core
F = 64  # features
B = 7  # tiles (node-rows) per matmul: N = 7*64 = 448 <= 512 f32 PSUM bank
TB = 63  # tiles per full DMA chunk (~1.03 MB per chunk)

# set by tests to capture a profile; harness path leaves these alone
TRACE = False
LAST_RESULTS = None


def _chunks(t_cap):
    """Split t_cap tiles into DMA chunks: full 63-tile chunks + remainder."""
    out = []
    t = 0
    while t < t_cap:
        n = min(TB, t_cap - t)
        out.append((t, n))
        t += n
    return out


def _build(t_cap):
    nc = bacc.Bacc("TRN2", target_bir_lowering=False, debug=False, num_devices=NCORES)
    hi = nc.dram_tensor("hi", [P, t_cap * F], mybir.dt.float16, kind="ExternalInput").ap()
    lo = nc.dram_tensor("lo", [P, t_cap * F], mybir.dt.float16, kind="ExternalInput").ap()
    ident = nc.dram_tensor("ident", [P, P], mybir.dt.float16, kind="ExternalInput").ap()
    inv = nc.dram_tensor("inv", [P, 1], mybir.dt.float32, kind="ExternalInput").ap()
    out = nc.dram_tensor("out", [P, F], mybir.dt.float32, kind="ExternalOutput").ap()

    chunks = _chunks(t_cap)
    n_mm = 2 * (t_cap // B)
    with tile.TileContext(nc) as tc:
        with (
            tc.tile_pool(name="consts", bufs=1) as consts,
            tc.tile_pool(name="io", bufs=4) as io,
            tc.tile_pool(name="ep", bufs=1) as ep,
            tc.tile_pool(name="acc", bufs=1, space="PSUM") as accp,
        ):
            ident_sb = consts.tile([P, P], mybir.dt.float16)
            nc.sync.dma_start(ident_sb[:], ident[:])
            inv_sb = consts.tile([P, 1], mybir.dt.float32)
            nc.sync.dma_start(inv_sb[:], inv[:])

            psum = accp.tile([P, B * F], mybir.dt.float32)
            mm = 0
            for t0, nt in chunks:
                hi_t = io.tile([P, TB * F], mybir.dt.float16, tag="hi")
                nc.sync.dma_start(hi_t[:, : nt * F], hi[:, t0 * F : (t0 + nt) * F])
                lo_t = io.tile([P, TB * F], mybir.dt.float16, tag="lo")
                # second HWDGE ring (ACT engine) so hi/lo issue in parallel
                nc.scalar.dma_start(lo_t[:, : nt * F], lo[:, t0 * F : (t0 + nt) * F])
                for t in (hi_t, lo_t):
                    for b in range(nt // B):
                        nc.tensor.matmul(
                            psum[:],
                            ident_sb[:],
                            t[:, b * B * F : (b + 1) * B * F],
                            start=(mm == 0),
                            stop=(mm == n_mm - 1),
                        )
                        mm += 1
            assert mm == n_mm

            # fold the B column blocks: s = sum_b psum[:, b*64:(b+1)*64]
            # (DVE may read at most one PSUM operand per instruction)
            s = ep.tile([P, F], mybir.dt.float32)
            nc.vector.tensor_copy(s[:], psum[:, 0:F])
            for b in range(1, B):
                nc.vector.tensor_add(s[:], s[:], psum[:, b * F : (b + 1) * F])

            res = ep.tile([P, F], mybir.dt.float32)
            nc.vector.tensor_scalar_mul(res[:], s[:], inv_sb[:])
            nc.sync.dma_start(out[:], res[:])

    nc.compile()
    return nc


def kernel(node_features, batch, num_graphs):
    global LAST_RESULTS
    x = np.asarray(node_features, dtype=np.float32)
    b = np.asarray(batch, dtype=np.int64).ravel()
    G = int(num_graphs)
    N = x.shape[0]
    assert x.shape[1] == F, f"expected {F} features, got {x.shape[1]}"

    if not np.all(b[1:] >= b[:-1]):  # defensive: layout relies on sorted batch
        order = np.argsort(b, kind="stable")
        b = b[order]
        x = x[order]

    gpc = math.ceil(G / NCORES)  # local graphs per core
    assert gpc <= P, f"num_graphs {G} too large for {NCORES} cores x {P} partitions"

    counts = np.bincount(b, minlength=NCORES * gpc).astype(np.int64)
    starts = np.zeros(NCORES * gpc + 1, dtype=np.int64)
    np.cumsum(counts, out=starts[1:])
    t_max = int(counts.max()) if N else 1
    t_cap = max(B, math.ceil(t_max / B) * B)

    x_ext = np.vstack([x, np.zeros((1, F), dtype=np.float32)])  # row N = zeros
    col = np.arange(t_cap, dtype=np.int64)
    ident = np.eye(P, dtype=np.float16)

    in_maps = []
    for k in range(NCORES):
        g0 = k * gpc
        cg = counts[g0 : g0 + gpc]
        sg = starts[g0 : g0 + gpc]
        valid = col[None, :] < cg[:, None]  # [gpc, t_cap]
        idx = np.where(valid, sg[:, None] + col[None, :], N)
        if gpc < P:  # pad partitions when graph count is not divisible by 8
            idx = np.vstack([idx, np.full((P - gpc, t_cap), N, dtype=np.int64)])

        feats = x_ext[idx]  # [P, t_cap, F] f32
        hi16 = np.ascontiguousarray(feats.astype(np.float16).reshape(P, t_cap * F))
        lo16 = np.ascontiguousarray(
            (feats - hi16.reshape(P, t_cap, F).astype(np.float32))
            .astype(np.float16)
            .reshape(P, t_cap * F)
        )

        inv = np.zeros((P, 1), dtype=np.float32)
        inv[:gpc, 0] = 1.0 / np.maximum(cg, 1)
        in_maps.append({"hi": hi16, "lo": lo16, "ident": ident, "inv": inv})

    nc = _build(t_cap)
    res = run_bass_kernel_spmd(nc, in_maps, core_ids=list(range(NCORES)), trace=TRACE)
    LAST_RESULTS = res

    out = np.concatenate([res.results[k]["out"] for k in range(NCORES)], axis=0)
    return out[:G]
